# revision 21
# baseline (speedup 1.0000x reference)
"""DSGCN block kernel for 8 Trainium2 NeuronCores.

Math notes (derived from the reference):
  - einsum('knm,btnc->kbtnc', A_eff, x) sums m ONLY within A, so
    agg[k,b,t,n,c] = S[k,n] * x[b,t,n,c] with S = rowsum(A_eff).
  - The whole pointwise stage collapses to a per-node GEMM:
      h[b,t,n,o] = sum_c x[b,t,n,c] * V[n,o,c],
      V[n] = sum_k S[k,n] * (dw[k,:] * W_pw[:, k*C:k*C+C])
  - Temporal depthwise conv folds into the GEMM by expanding the
    contraction over (dt, c) with V3[n,dt,o,c] = conv_w[o,dt]*V[n,o,c]
    and t-shifted views of x^T.
  - Sharding: nodes (N=47) split across 8 cores (6,6,...,5+1 dummy pad).
    All of (b, t) stays local per node -> conv/GN/LN fully local.

Device layout ("layout A"): per (node, b) block the GEMM produces
psum[128t, 0:256]=conv(h), [256:512]=residual. LayerNorm is per-row
(per-partition) so LN-apply + exact GELU fuse into one ScalarE
activation. GroupNorm stats via bn_stats + cross-partition ones-matmul.
"""

import numpy as np

import concourse.bass as bass
import concourse.bacc as bacc
import concourse.tile as tile
from concourse import mybir
from concourse.bass_utils import run_bass_kernel_spmd

B, T, N, C_IN, C_OUT, KADJ, KT, G = 32, 128, 47, 192, 256, 3, 3, 8
EPS = 1e-5
NCORES = 8
NN = 6            # node slots per core (core 7: 5 real + 1 dummy)
GS = C_OUT // G   # 32 channels per group
NB = B            # blocks per node = B (each block is [T=128 rows, ...])
GRP = 3           # blocks per stats group (PSUM budget: 2*3 main + 2 stats)
F32 = mybir.dt.float32
F32R = mybir.dt.float32r
I32 = mybir.dt.int32
AL = mybir.AluOpType
AF = mybir.ActivationFunctionType
RSQRT_MAGIC = 0x5F3759DF


def _emit_rsqrt(nc, eng, pool, u, full_shape, sl, tag_prefix):
    """rsqrt(u) via bit-trick seed + 1 Newton iter (max rel err ~1.8e-3).

    u must be strictly positive and well above denormal (here: 4096*var or
    65536*var, so ~O(1e2..1e6)). u must be SBUF. `eng` picks the engine
    (nc.vector or nc.gpsimd). `sl` slices each full tile down to the active
    region matching u. Returns the full rp tile.
    """
    iv = pool.tile(full_shape, I32, tag=f"{tag_prefix}iv")
    nc.vector.tensor_scalar(sl(iv), u.bitcast(I32), 1, None, AL.logical_shift_right)
    iv2 = pool.tile(full_shape, I32, tag=f"{tag_prefix}iv2")
    nc.vector.tensor_scalar(sl(iv2), sl(iv), -1, RSQRT_MAGIC, AL.mult, AL.add)
    s0 = sl(iv2).bitcast(F32)
    yy = pool.tile(full_shape, F32, tag=f"{tag_prefix}yy")
    eng.tensor_tensor(sl(yy), s0, s0, AL.mult)
    vyy = pool.tile(full_shape, F32, tag=f"{tag_prefix}vyy")
    eng.tensor_tensor(sl(vyy), u, sl(yy), AL.mult)
    half = pool.tile(full_shape, F32, tag=f"{tag_prefix}half")
    nc.vector.tensor_scalar(sl(half), sl(vyy), -0.5, 1.5, AL.mult, AL.add)
    rp = pool.tile(full_shape, F32, tag=f"{tag_prefix}rp")
    eng.tensor_tensor(sl(rp), s0, sl(half), AL.mult)
    return rp

_CACHE = {}
LAST_RUN_S = None


def _build(trivial_gn, trivial_ln):
    nc = bacc.Bacc()
    x_t = nc.dram_tensor("x_t", [NN, C_IN, B, T + 2], F32R, kind="ExternalInput")
    v3a = nc.dram_tensor("v3a", [NN, 128, KT, C_OUT], F32R, kind="ExternalInput")
    v3b = nc.dram_tensor("v3b", [NN, 64, KT, C_OUT], F32R, kind="ExternalInput")
    v3sa = nc.dram_tensor("v3sa", [NN, 128, KT, G], F32R, kind="ExternalInput")
    v3sb = nc.dram_tensor("v3sb", [NN, 64, KT, G], F32R, kind="ExternalInput")
    wra = nc.dram_tensor("wra", [128, C_OUT], F32R, kind="ExternalInput")
    wrb = nc.dram_tensor("wrb", [64, C_OUT], F32R, kind="ExternalInput")
    gng = nc.dram_tensor("gng", [1, C_OUT], F32, kind="ExternalInput")
    gnb = nc.dram_tensor("gnb", [1, C_OUT], F32, kind="ExternalInput")
    lng = nc.dram_tensor("lng", [1, C_OUT], F32, kind="ExternalInput")
    lnb = nc.dram_tensor("lnb", [1, C_OUT], F32, kind="ExternalInput")
    out_t = nc.dram_tensor("out_t", [NN, B, T, C_OUT], F32, kind="ExternalOutput")

    xav = x_t[:, 0:128]
    xbv = x_t[:, 128:192]

    with tile.TileContext(nc) as tc:
        with (
            tc.tile_pool(name="xp", bufs=3) as xp,
            tc.tile_pool(name="wp", bufs=2) as wp,
            tc.tile_pool(name="cst", bufs=1) as cst,
            tc.tile_pool(name="yb", bufs=9) as yb,
            tc.tile_pool(name="ob", bufs=2) as ob,
            tc.tile_pool(name="st", bufs=6) as st,
            tc.tile_pool(name="ps", bufs=6, space="PSUM") as ps,
            tc.tile_pool(name="pst", bufs=2, space="PSUM") as pst,
        ):
            ones_c = cst.tile([128, 1], F32)
            nc.vector.memset(ones_c, 1.0)
            ones_rf = cst.tile([1, 128], F32)
            nc.vector.memset(ones_rf, 1.0)
            ones_r = cst.tile([1, 128], F32R)
            nc.vector.tensor_copy(ones_r, ones_rf)
            eps1 = cst.tile([1, 1], F32)
            nc.vector.memset(eps1, EPS)
            epsl = cst.tile([128, 1], F32)
            nc.vector.memset(epsl, EPS)

            wra_s = cst.tile([128, C_OUT], F32R)
            nc.sync.dma_start(out=wra_s, in_=wra[:, :])
            wrb_s = cst.tile([64, C_OUT], F32R)
            nc.sync.dma_start(out=wrb_s, in_=wrb[:, :])

            if not trivial_gn:
                gng_t = cst.tile([128, C_OUT], F32)
                nc.gpsimd.dma_start(out=gng_t, in_=gng.broadcast_to([128, C_OUT]))
                gng_r = cst.tile([1, C_OUT], F32)
                nc.sync.dma_start(out=gng_r, in_=gng[:, :])
                gnb_r = cst.tile([1, C_OUT], F32)
                nc.sync.dma_start(out=gnb_r, in_=gnb[:, :])
            if not trivial_ln:
                lng_t = cst.tile([128, C_OUT], F32)
                nc.gpsimd.dma_start(out=lng_t, in_=lng.broadcast_to([128, C_OUT]))
                lnb_t = cst.tile([128, C_OUT], F32)
                nc.gpsimd.dma_start(out=lnb_t, in_=lnb.broadcast_to([128, C_OUT]))

            for n in range(NN):
                xa = xp.tile([128, B, T + 2], F32R, tag="xa")
                xb = xp.tile([64, B, T + 2], F32R, tag="xb")
                nc.sync.dma_start(out=xa, in_=xav[n])
                nc.sync.dma_start(out=xb, in_=xbv[n])

                va = wp.tile([128, KT, C_OUT], F32R, tag="va")
                nc.sync.dma_start(out=va, in_=v3a[n])
                vb = wp.tile([64, KT, C_OUT], F32R, tag="vb")
                nc.sync.dma_start(out=vb, in_=v3b[n])
                vsa = wp.tile([128, KT, G], F32R, tag="vsa")
                nc.sync.dma_start(out=vsa, in_=v3sa[n])
                vsb = wp.tile([64, KT, G], F32R, tag="vsb")
                nc.sync.dma_start(out=vsb, in_=v3sb[n])

                for g0 in range(0, NB, GRP):
                    gn_blocks = list(range(g0, min(g0 + GRP, NB)))
                    ng = len(gn_blocks)
                    mains = []
                    sums2 = st.tile([128, GRP, G, 2], F32, tag="sums2")
                    pstt = pst.tile([128, 72], F32, tag="pstt")
                    gp = pstt[:, 0 : GRP * G].rearrange("p (j g) -> p j g", g=G)
                    for j, b in enumerate(gn_blocks):
                        main = ps.tile([128, 512], F32, tag="main")
                        mains.append(main)
                        hcv = main[:, 0:C_OUT]
                        for dt in range(KT):
                            nc.tensor.matmul(
                                hcv,
                                xa[:, b, dt : dt + 128],
                                va[:, dt, :],
                                start=(dt == 0),
                                stop=False,
                            )
                        for dt in range(KT):
                            nc.tensor.matmul(
                                hcv,
                                xb[0:64, b, dt : dt + 128],
                                vb[0:64, dt, :],
                                start=False,
                                stop=(dt == KT - 1),
                            )
                        resv = main[:, C_OUT:512]
                        nc.tensor.matmul(
                            resv,
                            xa[:, b, 1 : T + 1],
                            wra_s[:, :],
                            start=True,
                            stop=False,
                        )
                        nc.tensor.matmul(
                            resv,
                            xb[0:64, b, 1 : T + 1],
                            wrb_s[:, :],
                            start=False,
                            stop=False,
                            skip_group_check=True,
                        )
                        # GroupNorm per-(t,g) sums via group-sum weight GEMM
                        # (replaces a DVE reduce of hcv)
                        for dt in range(KT):
                            nc.tensor.matmul(
                                gp[:, j],
                                xa[:, b, dt : dt + 128],
                                vsa[:, dt, :],
                                start=(dt == 0),
                                stop=False,
                            )
                        for dt in range(KT):
                            nc.tensor.matmul(
                                gp[:, j],
                                xb[0:64, b, dt : dt + 128],
                                vsb[0:64, dt, :],
                                start=False,
                                stop=(dt == KT - 1),
                            )
                        sq_sb = yb.tile([128, C_OUT], F32, tag="sqsb")
                        nc.scalar.activation(sq_sb, hcv, AF.Square)
                        nc.vector.tensor_reduce(
                            sums2[:, j, :, 1:2],
                            sq_sb.rearrange("p (g d) -> p g d", g=G),
                            mybir.AxisListType.X,
                            AL.add,
                        )
                    nc.vector.tensor_copy(sums2[:, 0:ng, :, 0], gp[:, 0:ng])

                    # cross-partition (t) reduce via ones-matmul
                    spsum = pstt[0:1, GRP * G : GRP * G * 3]
                    nc.tensor.matmul(
                        spsum[:, 0 : ng * G * 2],
                        ones_c[:, :],
                        sums2[:, 0:ng, :, :],
                        start=True,
                        stop=True,
                    )
                    spv = spsum[0:1, 0 : ng * G * 2].rearrange(
                        "p (j g d) -> p j g d", g=G, d=2
                    )
                    s1 = spv[:, :, :, 0:1]
                    s2 = spv[:, :, :, 1:2]
                    # m = s1/64 (PSUM->SBUF), then m^2 = 4096*mu^2
                    mcol = st.tile([1, GRP, G, 1], F32, tag="mcol")
                    nc.vector.tensor_scalar(mcol[:, 0:ng], s1, 1.0 / 64.0, None, AL.mult)
                    msq = st.tile([1, GRP, G, 1], F32, tag="msq")
                    nc.gpsimd.tensor_tensor(msq[:, 0:ng], mcol[:, 0:ng], mcol[:, 0:ng], AL.mult)
                    # 4096*var = s2 - 4096*mu^2  (eps is negligible: var >> 1e-5)
                    v4096 = st.tile([1, GRP, G, 1], F32, tag="v4096")
                    nc.vector.scalar_tensor_tensor(
                        v4096[:, 0:ng], msq[:, 0:ng], -1.0,
                        s2, AL.mult, AL.add,
                    )
                    # rstd = 64 * rsqrt(4096*var), Newton on Pool (no ACT Sqrt)
                    rpg = _emit_rsqrt(
                        nc, nc.gpsimd, st, v4096[:, 0:ng], [1, GRP, G, 1],
                        lambda t: t[:, 0:ng], "g",
                    )
                    rstd = st.tile([1, GRP, G, 1], F32, tag="rstd")
                    nc.vector.tensor_scalar(rstd[:, 0:ng], rpg[:, 0:ng], 64.0, None, AL.mult)
                    tg = st.tile([1, GRP, G, 1], F32R, tag="tg")
                    nc.vector.scalar_tensor_tensor(
                        tg[:, 0:ng], s1, -1.0 / 64.0, rpg[:, 0:ng],
                        AL.mult, AL.mult,
                    )
                    # replicate scale across partitions; expand bias to [1,256] rows
                    sbc = st.tile([128, GRP, G], F32, tag="sbc")
                    nc.gpsimd.partition_broadcast(
                        sbc[:, 0:ng, :], rstd[0:1, 0:ng, :, 0]
                    )
                    if not trivial_gn:
                        # t_full = t' * gamma + beta ; applied via K=1 matmul
                        trows = st.tile([1, GS, GRP * G], F32R, tag="trows")
                        nc.gpsimd.dma_start(
                            out=trows[:, :, 0 : ng * G],
                            in_=tg[0:1, 0:ng, :, 0]
                            .rearrange("p j g -> p (j g)")
                            .unsqueeze(1)
                            .broadcast_to([1, GS, ng * G]),
                        )
                        trows2 = st.tile([1, GS, GRP * G], F32R, tag="trows2")
                        for j in range(ng):
                            tv = trows[0:1, :, j * G : (j + 1) * G].transpose([0, 2, 1])
                            tv2 = trows2[0:1, :, j * G : (j + 1) * G].transpose([0, 2, 1])
                            nc.vector.tensor_tensor(
                                tv2, tv, gng_r.rearrange("p (g d) -> p g d", g=G),
                                AL.mult,
                            )
                            nc.vector.tensor_tensor(
                                tv2, tv2, gnb_r.rearrange("p (g d) -> p g d", g=G),
                                AL.add,
                            )
                        trows = trows2

                    lnacc = st.tile([128, GRP, 2], F32, tag="lnacc")
                    ys = []
                    for j, b in enumerate(gn_blocks):
                        main = mains[j]
                        hcv = main[:, 0:C_OUT]
                        resv = main[:, C_OUT:512]
                        # fold GN bias into residual (PSUM accumulate, K=1);
                        # trivial path reads tg directly via stride-0 AP
                        if trivial_gn:
                            fold_mov = tg[0:1, j].broadcast_to([1, G, GS])
                        else:
                            fold_mov = trows[0:1, :, j * G : (j + 1) * G].transpose([0, 2, 1])
                        nc.tensor.matmul(
                            resv,
                            ones_r[:, :],
                            fold_mov,
                            start=False,
                            stop=True,
                            skip_group_check=True,
                        )
                        # y1 = hc * rstd (broadcast over t partitions and 32-chans)
                        y1 = yb.tile([128, C_OUT], F32, tag="y1")
                        sview = (
                            sbc[:, j, :]
                            .unsqueeze(-1)
                            .broadcast_to([128, G, GS])
                        )
                        nc.vector.tensor_tensor(
                            y1.rearrange("p (g d) -> p g d", g=G),
                            hcv.rearrange("p (g d) -> p g d", g=G),
                            sview,
                            AL.mult,
                        )
                        if not trivial_gn:
                            nc.vector.tensor_tensor(y1, y1, gng_t[:, :], AL.mult)
                        # y = y1 + res' ; accum -> LN row sums
                        y = yb.tile([128, C_OUT], F32, tag="y")
                        nc.vector.scalar_tensor_tensor(
                            y, y1, 1.0, resv, AL.mult, AL.add,
                            accum_out=lnacc[:, j, 0:1],
                        )
                        ys.append(y)
                        # LN sum of squares via ACT Square (result discarded)
                        sq2 = yb.tile([128, C_OUT], F32, tag="sq2")
                        nc.scalar.activation(
                            sq2, y, AF.Square, accum_out=lnacc[:, j, 1:2]
                        )
                    # LN finalize (batched per group; all-SBUF -> Pool engine)
                    lnsum = lnacc[:, 0:ng, 0:1]
                    lnsq = lnacc[:, 0:ng, 1:2]
                    nsq = st.tile([128, GRP, 1], F32, tag="nsq")
                    nc.vector.scalar_tensor_tensor(
                        nsq[:, 0:ng], lnsum, -1.0, lnsum, AL.mult, AL.mult
                    )
                    vln = st.tile([128, GRP, 1], F32, tag="vln")
                    nc.vector.scalar_tensor_tensor(
                        vln[:, 0:ng], lnsq, float(C_OUT), nsq[:, 0:ng],
                        AL.mult, AL.add,
                    )
                    # rstd_ln = 256 * rsqrt(65536*var), Newton on Pool
                    rpl = _emit_rsqrt(
                        nc, nc.gpsimd, st, vln[:, 0:ng], [128, GRP, 1],
                        lambda t: t[:, 0:ng], "l",
                    )
                    acol = st.tile([128, GRP, 1], F32, tag="acol")
                    nc.vector.tensor_scalar(acol[:, 0:ng], rpl[:, 0:ng], 256.0, None, AL.mult)
                    bcol = st.tile([128, GRP, 1], F32, tag="bcol")
                    nc.vector.scalar_tensor_tensor(
                        bcol[:, 0:ng], lnsum, -1.0 / float(C_OUT), acol[:, 0:ng],
                        AL.mult, AL.mult,
                    )
                    outg = ob.tile([128, GRP, C_OUT], F32, tag="outg")
                    for j, b in enumerate(gn_blocks):
                        if trivial_ln:
                            nc.scalar.activation(
                                outg[:, j], ys[j], AF.Gelu,
                                bias=bcol[:, j], scale=acol[:, j],
                            )
                        else:
                            z = yb.tile([128, C_OUT], F32, tag="z")
                            nc.scalar.activation(
                                z, ys[j], AF.Identity,
                                bias=bcol[:, j], scale=acol[:, j],
                            )
                            nc.vector.tensor_tensor(z, z, lng_t[:, :], AL.mult)
                            nc.vector.tensor_tensor(z, z, lnb_t[:, :], AL.add)
                            nc.scalar.activation(outg[:, j], z, AF.Gelu)
                    nc.sync.dma_start(
                        out=out_t[n, g0 : g0 + ng].transpose([1, 0, 2]),
                        in_=outg[:, 0:ng],
                    )
    nc.finalize()
    return nc


def kernel(**inputs):
    x = np.asarray(inputs["x"], np.float32)
    A = np.asarray(inputs["A"], np.float32)
    dw = np.asarray(inputs["dw_weights"], np.float32)
    adjr = np.asarray(inputs["adj_residual"], np.float32)
    W_pw = np.asarray(inputs["W_pw"], np.float32)
    conv_w = np.asarray(inputs["conv_w"], np.float32)
    gng = np.asarray(inputs["gn_gamma"], np.float32)
    gnb = np.asarray(inputs["gn_beta"], np.float32)
    lng = np.asarray(inputs["ln_gamma"], np.float32)
    lnb = np.asarray(inputs["ln_beta"], np.float32)
    W_res = np.asarray(inputs["W_res"], np.float32)

    # ---- tiny host precompute (replicated params only) ----
    A_eff = A + np.tanh(adjr) * 0.3
    A_eff = A_eff / np.clip(np.abs(A_eff).sum(-1, keepdims=True), 1.0, None)
    S = A_eff.sum(-1)                                   # (K, N)
    Wk = W_pw.reshape(C_OUT, KADJ, C_IN).transpose(1, 0, 2) * dw[:, None, :]
    V = np.einsum("kn,koc->noc", S, Wk)                  # (N, C_OUT, C_IN)
    V3 = conv_w[None, :, 0, :, None] * V[:, :, None, :]  # (N, O, KT, C)
    V3 = V3.transpose(0, 3, 2, 1).copy()                 # (N, C, KT, O)
    Vsum3 = V3.reshape(N, C_IN, KT, G, C_OUT // G).sum(-1)  # (N, C, KT, G)
    Vsum3 = np.ascontiguousarray(Vsum3)
    WrT = np.ascontiguousarray(W_res.T)                  # (C, O)

    trivial_gn = bool(np.all(gng == 1.0) and np.all(gnb == 0.0))
    trivial_ln = bool(np.all(lng == 1.0) and np.all(lnb == 0.0))

    key = (trivial_gn, trivial_ln)
    if key not in _CACHE:
        _CACHE[key] = _build(*key)
    nc = _CACHE[key]

    # ---- shard nodes across cores ----
    splits = [6, 6, 6, 6, 6, 6, 6, 5]
    starts = np.cumsum([0] + splits[:-1])
    xt_full = np.zeros((N, C_IN, B, T + 2), np.float32)      # zero-padded t
    xt_full[:, :, :, 1 : T + 1] = x.transpose(2, 3, 0, 1)
    in_maps = []
    for c in range(NCORES):
        n0, nn = starts[c], splits[c]
        idx = list(range(n0, n0 + nn)) + [0] * (NN - nn)
        in_maps.append({
            "x_t": np.ascontiguousarray(xt_full[idx]),
            "v3a": np.ascontiguousarray(V3[idx, 0:128]),
            "v3b": np.ascontiguousarray(V3[idx, 128:192]),
            "v3sa": np.ascontiguousarray(Vsum3[idx, 0:128]),
            "v3sb": np.ascontiguousarray(Vsum3[idx, 128:192]),
            "wra": np.ascontiguousarray(WrT[0:128]),
            "wrb": np.ascontiguousarray(WrT[128:192]),
            "gng": gng.reshape(1, -1).copy(),
            "gnb": gnb.reshape(1, -1).copy(),
            "lng": lng.reshape(1, -1).copy(),
            "lnb": lnb.reshape(1, -1).copy(),
        })

    import time as _time
    _t0 = _time.perf_counter()
    res = run_bass_kernel_spmd(nc, in_maps, core_ids=list(range(NCORES)))
    global LAST_RUN_S
    LAST_RUN_S = _time.perf_counter() - _t0
    out = np.empty((B, T, N, C_OUT), np.float32)
    for c in range(NCORES):
        n0, nn = starts[c], splits[c]
        o = res.results[c]["out_t"]  # (NN, B, T, C_OUT)
        out[:, :, n0 : n0 + nn, :] = o[:nn].transpose(1, 2, 0, 3)
    return out



# revision 41
# speedup vs baseline: 1.0231x; 1.0231x over previous
"""DSGCN block kernel for 8 Trainium2 NeuronCores.

Math notes (derived from the reference):
  - einsum('knm,btnc->kbtnc', A_eff, x) sums m ONLY within A, so
    agg[k,b,t,n,c] = S[k,n] * x[b,t,n,c] with S = rowsum(A_eff).
  - The whole pointwise stage collapses to a per-node GEMM:
      h[b,t,n,o] = sum_c x[b,t,n,c] * V[n,o,c],
      V[n] = sum_k S[k,n] * (dw[k,:] * W_pw[:, k*C:k*C+C])
  - Temporal depthwise conv folds into the GEMM by expanding the
    contraction over (dt, c) with V3[n,dt,o,c] = conv_w[o,dt]*V[n,o,c]
    and t-shifted views of x^T.
  - Sharding: nodes (N=47) split across 8 cores (6,6,...,5+1 dummy pad).
    All of (b, t) stays local per node -> conv/GN/LN fully local.

Device layout ("layout A"): per (node, b) block the GEMM produces
psum[128t, 0:256]=conv(h), [256:512]=residual. LayerNorm is per-row
(per-partition) so LN-apply + exact GELU fuse into one ScalarE
activation. GroupNorm stats via bn_stats + cross-partition ones-matmul.
"""

import numpy as np

import concourse.bass as bass
import concourse.bacc as bacc
import concourse.tile as tile
from concourse import mybir
from concourse.bass_utils import run_bass_kernel_spmd

B, T, N, C_IN, C_OUT, KADJ, KT, G = 32, 128, 47, 192, 256, 3, 3, 8
EPS = 1e-5
NCORES = 8
NN = 6            # node slots per core (core 7: 5 real + 1 dummy)
GS = C_OUT // G   # 32 channels per group
NB = B            # blocks per node = B (each block is [T=128 rows, ...])
GRP = 3           # blocks per stats group (PSUM budget: 2*3 main + 2 stats)
F32 = mybir.dt.float32
F32R = mybir.dt.float32r
I32 = mybir.dt.int32
AL = mybir.AluOpType
AF = mybir.ActivationFunctionType
RSQRT_MAGIC = 0x5F3759DF


def _emit_rsqrt(nc, eng, pool, u, full_shape, sl, tag_prefix):
    """rsqrt(u) via bit-trick seed + 1 Newton iter (max rel err ~1.8e-3).

    u must be strictly positive and well above denormal (here: 4096*var or
    65536*var, so ~O(1e2..1e6)). u must be SBUF. `eng` picks the engine
    (nc.vector or nc.gpsimd). `sl` slices each full tile down to the active
    region matching u. Returns the full rp tile.
    """
    iv = pool.tile(full_shape, I32, tag=f"{tag_prefix}iv")
    nc.vector.tensor_scalar(sl(iv), u.bitcast(I32), 1, None, AL.logical_shift_right)
    iv2 = pool.tile(full_shape, I32, tag=f"{tag_prefix}iv2")
    nc.vector.tensor_scalar(sl(iv2), sl(iv), -1, RSQRT_MAGIC, AL.mult, AL.add)
    s0 = sl(iv2).bitcast(F32)
    yy = pool.tile(full_shape, F32, tag=f"{tag_prefix}yy")
    eng.tensor_tensor(sl(yy), s0, s0, AL.mult)
    vyy = pool.tile(full_shape, F32, tag=f"{tag_prefix}vyy")
    eng.tensor_tensor(sl(vyy), u, sl(yy), AL.mult)
    half = pool.tile(full_shape, F32, tag=f"{tag_prefix}half")
    nc.vector.tensor_scalar(sl(half), sl(vyy), -0.5, 1.5, AL.mult, AL.add)
    rp = pool.tile(full_shape, F32, tag=f"{tag_prefix}rp")
    eng.tensor_tensor(sl(rp), s0, sl(half), AL.mult)
    return rp

_CACHE = {}
LAST_RUN_S = None


def _build(trivial_gn, trivial_ln):
    nc = bacc.Bacc()
    x_t = nc.dram_tensor("x_t", [NN, C_IN, B, T + 2], F32R, kind="ExternalInput")
    v3a = nc.dram_tensor("v3a", [NN, 128, KT, C_OUT], F32R, kind="ExternalInput")
    v3b = nc.dram_tensor("v3b", [NN, 64, KT, C_OUT], F32R, kind="ExternalInput")
    v3sa = nc.dram_tensor("v3sa", [NN, 128, KT, G], F32R, kind="ExternalInput")
    v3sb = nc.dram_tensor("v3sb", [NN, 64, KT, G], F32R, kind="ExternalInput")
    eye3d = nc.dram_tensor("eye3d", [GRP, GRP], F32R, kind="ExternalInput")
    wra = nc.dram_tensor("wra", [128, C_OUT], F32R, kind="ExternalInput")
    wrb = nc.dram_tensor("wrb", [64, C_OUT], F32R, kind="ExternalInput")
    gng = nc.dram_tensor("gng", [1, C_OUT], F32, kind="ExternalInput")
    gnb = nc.dram_tensor("gnb", [1, C_OUT], F32, kind="ExternalInput")
    lng = nc.dram_tensor("lng", [1, C_OUT], F32, kind="ExternalInput")
    lnb = nc.dram_tensor("lnb", [1, C_OUT], F32, kind="ExternalInput")
    out_t = nc.dram_tensor("out_t", [NN, B, T, C_OUT], F32, kind="ExternalOutput")

    xav = x_t[:, 0:128]
    xbv = x_t[:, 128:192]

    with tile.TileContext(nc) as tc:
        with (
            tc.tile_pool(name="xp", bufs=3) as xp,
            tc.tile_pool(name="wp", bufs=2) as wp,
            tc.tile_pool(name="cst", bufs=1) as cst,
            tc.tile_pool(name="yb", bufs=9) as yb,
            tc.tile_pool(name="ob", bufs=2) as ob,
            tc.tile_pool(name="st", bufs=6) as st,
            tc.tile_pool(name="ps", bufs=6, space="PSUM") as ps,
            tc.tile_pool(name="pst", bufs=1, space="PSUM") as pst,
        ):
            ones_c = cst.tile([128, 1], F32)
            nc.vector.memset(ones_c, 1.0)
            ones_cr = cst.tile([128, 1], F32R)
            nc.vector.tensor_copy(ones_cr, ones_c)
            # ez: zeros except column GRP-1; slice [GRP-1-j : 2*GRP-1-j]
            # gives a [128, GRP] one-hot-column-j stationary operand
            ezf = cst.tile([128, 2 * GRP - 1], F32)
            nc.vector.memset(ezf, 0.0)
            nc.vector.memset(ezf[:, GRP - 1 : GRP], 1.0)
            ez = cst.tile([128, 2 * GRP - 1], F32R)
            nc.vector.tensor_copy(ez, ezf)
            eye3 = cst.tile([GRP, GRP], F32R)
            nc.sync.dma_start(out=eye3, in_=eye3d[:, :])
            ones_rf = cst.tile([96, 128], F32)
            nc.vector.memset(ones_rf, 1.0)
            ones_r96 = cst.tile([96, 128], F32R)
            nc.vector.tensor_copy(ones_r96, ones_rf)
            ones_r = ones_r96[0:1, :]
            eps1 = cst.tile([1, 1], F32)
            nc.vector.memset(eps1, EPS)
            epsl = cst.tile([128, 1], F32)
            nc.vector.memset(epsl, EPS)

            wra_s = cst.tile([128, C_OUT], F32R)
            nc.sync.dma_start(out=wra_s, in_=wra[:, :])
            wrb_s = cst.tile([64, C_OUT], F32R)
            nc.sync.dma_start(out=wrb_s, in_=wrb[:, :])

            if not trivial_gn:
                gng_t = cst.tile([128, C_OUT], F32)
                nc.gpsimd.dma_start(out=gng_t, in_=gng.broadcast_to([128, C_OUT]))
                gng_r = cst.tile([1, C_OUT], F32)
                nc.sync.dma_start(out=gng_r, in_=gng[:, :])
                gnb_r = cst.tile([1, C_OUT], F32)
                nc.sync.dma_start(out=gnb_r, in_=gnb[:, :])
            if not trivial_ln:
                lng_t = cst.tile([128, C_OUT], F32)
                nc.gpsimd.dma_start(out=lng_t, in_=lng.broadcast_to([128, C_OUT]))
                lnb_t = cst.tile([128, C_OUT], F32)
                nc.gpsimd.dma_start(out=lnb_t, in_=lnb.broadcast_to([128, C_OUT]))

            for n in range(NN):
                xa = xp.tile([128, B, T + 2], F32R, tag="xa")
                xb = xp.tile([64, B, T + 2], F32R, tag="xb")
                nc.sync.dma_start(out=xa, in_=xav[n])
                nc.sync.dma_start(out=xb, in_=xbv[n])

                va = wp.tile([128, KT, C_OUT], F32R, tag="va")
                nc.sync.dma_start(out=va, in_=v3a[n])
                vb = wp.tile([64, KT, C_OUT], F32R, tag="vb")
                nc.sync.dma_start(out=vb, in_=v3b[n])
                vsa = wp.tile([128, KT, G], F32R, tag="vsa")
                nc.sync.dma_start(out=vsa, in_=v3sa[n])
                vsb = wp.tile([64, KT, G], F32R, tag="vsb")
                nc.sync.dma_start(out=vsb, in_=v3sb[n])

                for g0 in range(0, NB, GRP):
                    gn_blocks = list(range(g0, min(g0 + GRP, NB)))
                    ng = len(gn_blocks)
                    mains = []
                    sums2 = None if trivial_gn else st.tile([128, GRP, G, 2], F32, tag="sums2")
                    # sqraw needs its own bank: while its 3-matmul accum
                    # group is open, no other matmul may write the same bank
                    sqb = pst.tile([128, 256], F32, tag="sqb")
                    sqraw = sqb[0:GRP]
                    pstt = pst.tile([128, 88], F32, tag="pstt")
                    gp = pstt[:, 0:24].rearrange("p (j g) -> p j g", g=G)
                    for j, b in enumerate(gn_blocks):
                        main = ps.tile([128, 512], F32, tag="main")
                        mains.append(main)
                        hcv = main[:, 0:C_OUT]
                        for dt in range(KT):
                            nc.tensor.matmul(
                                hcv,
                                xa[:, b, dt : dt + 128],
                                va[:, dt, :],
                                start=(dt == 0),
                                stop=False,
                            )
                        for dt in range(KT):
                            nc.tensor.matmul(
                                hcv,
                                xb[0:64, b, dt : dt + 128],
                                vb[0:64, dt, :],
                                start=False,
                                stop=(dt == KT - 1),
                            )
                        resv = main[:, C_OUT:512]
                        nc.tensor.matmul(
                            resv,
                            xa[:, b, 1 : T + 1],
                            wra_s[:, :],
                            start=True,
                            stop=False,
                        )
                        nc.tensor.matmul(
                            resv,
                            xb[0:64, b, 1 : T + 1],
                            wrb_s[:, :],
                            start=False,
                            stop=False,
                            skip_group_check=True,
                        )
                        # GroupNorm per-(t,g) sums via group-sum weight GEMM
                        # (replaces a DVE reduce of hcv)
                        for dt in range(KT):
                            nc.tensor.matmul(
                                gp[:, j],
                                xa[:, b, dt : dt + 128],
                                vsa[:, dt, :],
                                start=(dt == 0),
                                stop=False,
                            )
                        for dt in range(KT):
                            nc.tensor.matmul(
                                gp[:, j],
                                xb[0:64, b, dt : dt + 128],
                                vsb[0:64, dt, :],
                                start=False,
                                stop=(dt == KT - 1),
                            )
                        sq_sb = yb.tile([128, C_OUT], F32R if trivial_gn else F32, tag="sqsb")
                        nc.scalar.activation(sq_sb, hcv, AF.Square)
                        if trivial_gn:
                            # per-block t-colsum of h^2 into psum row 32j (PE)
                            nc.tensor.matmul(
                                sqb[0:GRP, :],
                                ez[:, GRP - 1 - j : 2 * GRP - 1 - j],
                                sq_sb,
                                start=(j == 0), stop=(j == ng - 1),
                                skip_group_check=True,
                            )
                        else:
                            nc.vector.tensor_reduce(
                                sums2[:, j, :, 1:2],
                                sq_sb.rearrange("p (g d) -> p g d", g=G),
                                mybir.AxisListType.X,
                                AL.add,
                            )

                    if trivial_gn:
                        # s1 row: copy gsums to SBUF, ones-mm -> [1, ng*G]
                        sums2r = st.tile([128, GRP, G], F32R, tag="sums2r")
                        nc.vector.tensor_copy(sums2r[:, 0:ng], gp[:, 0:ng])
                        s1row = pstt[0:1, 32 : 32 + GRP * G]
                        nc.tensor.matmul(
                            s1row[:, 0 : ng * G], ones_cr, sums2r[:, 0:ng],
                            start=True, stop=True,
                        )
                        # s2: one shared group-reduce of the colsum rows, then
                        # partition-broadcast each row back to row form
                        s2sb = st.tile([GRP, G], F32R, tag="s2sb")
                        with nc.allow_low_precision(reason="f32r is f32-width"):
                            nc.vector.tensor_reduce(
                                s2sb[0:ng],
                                sqraw[0:ng].rearrange("p (g d) -> p g d", g=G),
                                mybir.AxisListType.X,
                                AL.add,
                            )
                        # row-extract each block's s2 via one-hot matmul
                        s2row = pstt[0:1, 64 : 64 + GRP * G]
                        for j in range(ng):
                            nc.tensor.matmul(
                                s2row[:, j * G : (j + 1) * G],
                                eye3[0:ng, j : j + 1],
                                s2sb[0:ng],
                                start=True, stop=True,
                            )
                        s1 = s1row.rearrange("p (j g) -> p j g", g=G).unsqueeze(-1)[:, 0:ng]
                        s2 = s2row.rearrange("p (j g) -> p j g", g=G).unsqueeze(-1)[:, 0:ng]
                        # m = -s1/64 (PSUM->SBUF), then m^2 = 4096*mu^2
                        mcol = st.tile([1, GRP, G, 1], F32, tag="mcol")
                        nc.vector.tensor_scalar(mcol[:, 0:ng], s1, -1.0 / 64.0, None, AL.mult)
                        msq = st.tile([1, GRP, G, 1], F32, tag="msq")
                        nc.gpsimd.tensor_tensor(msq[:, 0:ng], mcol[:, 0:ng], mcol[:, 0:ng], AL.mult)
                        v4096 = st.tile([1, GRP, G, 1], F32, tag="v4096")
                        nc.vector.scalar_tensor_tensor(
                            v4096[:, 0:ng], msq[:, 0:ng], -1.0,
                            s2, AL.mult, AL.add,
                        )
                        rpg = _emit_rsqrt(
                            nc, nc.gpsimd, st, v4096[:, 0:ng], [1, GRP, G, 1],
                            lambda t: t[:, 0:ng], "g",
                        )
                        rstd = st.tile([1, GRP, G, 1], F32, tag="rstd")
                        nc.vector.tensor_scalar(rstd[:, 0:ng], rpg[:, 0:ng], 64.0, None, AL.mult)
                        tg2 = st.tile([1, GRP, G, 1], F32R, tag="tg2")
                        nc.gpsimd.tensor_tensor(
                            tg2[:, 0:ng], mcol[:, 0:ng], rpg[:, 0:ng], AL.mult
                        )
                        sbc = st.tile([128, GRP, G], F32, tag="sbc")
                        nc.gpsimd.partition_broadcast(
                            sbc[:, 0:ng, :], rstd[0:1, 0:ng, :, 0]
                        )
                    else:
                        nc.vector.tensor_copy(sums2[:, 0:ng, :, 0], gp[:, 0:ng])
                        # cross-partition (t) reduce via ones-matmul
                        spsum = pstt[0:1, 24:72]
                        nc.tensor.matmul(
                            spsum[:, 0 : ng * G * 2],
                            ones_c[:, :],
                            sums2[:, 0:ng, :, :],
                            start=True,
                            stop=True,
                        )
                        spv = spsum[0:1, 0 : ng * G * 2].rearrange(
                            "p (j g d) -> p j g d", g=G, d=2
                        )
                        s1 = spv[:, :, :, 0:1]
                        s2 = spv[:, :, :, 1:2]
                        # m = s1/64 (PSUM->SBUF), then m^2 = 4096*mu^2
                        mcol = st.tile([1, GRP, G, 1], F32, tag="mcol")
                        nc.vector.tensor_scalar(mcol[:, 0:ng], s1, 1.0 / 64.0, None, AL.mult)
                        msq = st.tile([1, GRP, G, 1], F32, tag="msq")
                        nc.gpsimd.tensor_tensor(msq[:, 0:ng], mcol[:, 0:ng], mcol[:, 0:ng], AL.mult)
                        # 4096*var = s2 - 4096*mu^2  (eps is negligible)
                        v4096 = st.tile([1, GRP, G, 1], F32, tag="v4096")
                        nc.vector.scalar_tensor_tensor(
                            v4096[:, 0:ng], msq[:, 0:ng], -1.0,
                            s2, AL.mult, AL.add,
                        )
                        rpg = _emit_rsqrt(
                            nc, nc.gpsimd, st, v4096[:, 0:ng], [1, GRP, G, 1],
                            lambda t: t[:, 0:ng], "g",
                        )
                        rstd = st.tile([1, GRP, G, 1], F32, tag="rstd")
                        nc.vector.tensor_scalar(rstd[:, 0:ng], rpg[:, 0:ng], 64.0, None, AL.mult)
                        tg = st.tile([1, GRP, G, 1], F32R, tag="tg")
                        nc.vector.scalar_tensor_tensor(
                            tg[:, 0:ng], s1, -1.0 / 64.0, rpg[:, 0:ng],
                            AL.mult, AL.mult,
                        )
                        sbc = st.tile([128, GRP, G], F32, tag="sbc")
                        nc.gpsimd.partition_broadcast(
                            sbc[:, 0:ng, :], rstd[0:1, 0:ng, :, 0]
                        )
                        # t_full = t' * gamma + beta ; applied via K=1 matmul
                        trows = st.tile([1, GS, GRP * G], F32R, tag="trows")
                        nc.gpsimd.dma_start(
                            out=trows[:, :, 0 : ng * G],
                            in_=tg[0:1, 0:ng, :, 0]
                            .rearrange("p j g -> p (j g)")
                            .unsqueeze(1)
                            .broadcast_to([1, GS, ng * G]),
                        )
                        trows2 = st.tile([1, GS, GRP * G], F32R, tag="trows2")
                        for j in range(ng):
                            tv = trows[0:1, :, j * G : (j + 1) * G].transpose([0, 2, 1])
                            tv2 = trows2[0:1, :, j * G : (j + 1) * G].transpose([0, 2, 1])
                            nc.vector.tensor_tensor(
                                tv2, tv, gng_r.rearrange("p (g d) -> p g d", g=G),
                                AL.mult,
                            )
                            nc.vector.tensor_tensor(
                                tv2, tv2, gnb_r.rearrange("p (g d) -> p g d", g=G),
                                AL.add,
                            )
                        trows = trows2

                    lnacc = st.tile([128, GRP, 2], F32, tag="lnacc")
                    ys = []
                    for j, b in enumerate(gn_blocks):
                        main = mains[j]
                        hcv = main[:, 0:C_OUT]
                        resv = main[:, C_OUT:512]
                        # fold GN bias into residual (PSUM accumulate, K=1);
                        # trivial path reads tg directly via stride-0 AP
                        if trivial_gn:
                            fold_mov = tg2[0:1, j].broadcast_to([1, G, GS])
                        else:
                            fold_mov = trows[0:1, :, j * G : (j + 1) * G].transpose([0, 2, 1])
                        nc.tensor.matmul(
                            resv,
                            ones_r[:, :],
                            fold_mov,
                            start=False,
                            stop=True,
                            skip_group_check=True,
                        )
                        # y1 = hc * rstd (broadcast over t partitions and 32-chans)
                        y1 = yb.tile([128, C_OUT], F32, tag="y1")
                        sview = (
                            sbc[:, j, :]
                            .unsqueeze(-1)
                            .broadcast_to([128, G, GS])
                        )
                        nc.vector.tensor_tensor(
                            y1.rearrange("p (g d) -> p g d", g=G),
                            hcv.rearrange("p (g d) -> p g d", g=G),
                            sview,
                            AL.mult,
                        )
                        if not trivial_gn:
                            nc.vector.tensor_tensor(y1, y1, gng_t[:, :], AL.mult)
                        # y = y1 + res' ; accum -> LN row sums
                        y = yb.tile([128, C_OUT], F32, tag="y")
                        nc.vector.scalar_tensor_tensor(
                            y, y1, 1.0, resv, AL.mult, AL.add,
                            accum_out=lnacc[:, j, 0:1],
                        )
                        ys.append(y)
                        # LN sum of squares via ACT Square (result discarded)
                        sq2 = yb.tile([128, C_OUT], F32, tag="sq2")
                        nc.scalar.activation(
                            sq2, y, AF.Square, accum_out=lnacc[:, j, 1:2]
                        )
                    # LN finalize (batched per group; all-SBUF -> Pool engine)
                    lnsum = lnacc[:, 0:ng, 0:1]
                    lnsq = lnacc[:, 0:ng, 1:2]
                    nsq = st.tile([128, GRP, 1], F32, tag="nsq")
                    nc.vector.scalar_tensor_tensor(
                        nsq[:, 0:ng], lnsum, -1.0, lnsum, AL.mult, AL.mult
                    )
                    vln = st.tile([128, GRP, 1], F32, tag="vln")
                    nc.vector.scalar_tensor_tensor(
                        vln[:, 0:ng], lnsq, float(C_OUT), nsq[:, 0:ng],
                        AL.mult, AL.add,
                    )
                    # rstd_ln = 256 * rsqrt(65536*var), Newton on Pool
                    rpl = _emit_rsqrt(
                        nc, nc.gpsimd, st, vln[:, 0:ng], [128, GRP, 1],
                        lambda t: t[:, 0:ng], "l",
                    )
                    acol = st.tile([128, GRP, 1], F32, tag="acol")
                    nc.vector.tensor_scalar(acol[:, 0:ng], rpl[:, 0:ng], 256.0, None, AL.mult)
                    bcol = st.tile([128, GRP, 1], F32, tag="bcol")
                    nc.vector.scalar_tensor_tensor(
                        bcol[:, 0:ng], lnsum, -1.0 / float(C_OUT), acol[:, 0:ng],
                        AL.mult, AL.mult,
                    )
                    outg = ob.tile([128, GRP, C_OUT], F32, tag="outg")
                    for j, b in enumerate(gn_blocks):
                        if trivial_ln:
                            nc.scalar.activation(
                                outg[:, j], ys[j], AF.Gelu,
                                bias=bcol[:, j], scale=acol[:, j],
                            )
                        else:
                            z = yb.tile([128, C_OUT], F32, tag="z")
                            nc.scalar.activation(
                                z, ys[j], AF.Identity,
                                bias=bcol[:, j], scale=acol[:, j],
                            )
                            nc.vector.tensor_tensor(z, z, lng_t[:, :], AL.mult)
                            nc.vector.tensor_tensor(z, z, lnb_t[:, :], AL.add)
                            nc.scalar.activation(outg[:, j], z, AF.Gelu)
                    nc.sync.dma_start(
                        out=out_t[n, g0 : g0 + ng].transpose([1, 0, 2]),
                        in_=outg[:, 0:ng],
                    )
    nc.finalize()
    return nc


def kernel(**inputs):
    x = np.asarray(inputs["x"], np.float32)
    A = np.asarray(inputs["A"], np.float32)
    dw = np.asarray(inputs["dw_weights"], np.float32)
    adjr = np.asarray(inputs["adj_residual"], np.float32)
    W_pw = np.asarray(inputs["W_pw"], np.float32)
    conv_w = np.asarray(inputs["conv_w"], np.float32)
    gng = np.asarray(inputs["gn_gamma"], np.float32)
    gnb = np.asarray(inputs["gn_beta"], np.float32)
    lng = np.asarray(inputs["ln_gamma"], np.float32)
    lnb = np.asarray(inputs["ln_beta"], np.float32)
    W_res = np.asarray(inputs["W_res"], np.float32)

    # ---- tiny host precompute (replicated params only) ----
    A_eff = A + np.tanh(adjr) * 0.3
    A_eff = A_eff / np.clip(np.abs(A_eff).sum(-1, keepdims=True), 1.0, None)
    S = A_eff.sum(-1)                                   # (K, N)
    Wk = W_pw.reshape(C_OUT, KADJ, C_IN).transpose(1, 0, 2) * dw[:, None, :]
    V = np.einsum("kn,koc->noc", S, Wk)                  # (N, C_OUT, C_IN)
    V3 = conv_w[None, :, 0, :, None] * V[:, :, None, :]  # (N, O, KT, C)
    V3 = V3.transpose(0, 3, 2, 1).copy()                 # (N, C, KT, O)
    Vsum3 = V3.reshape(N, C_IN, KT, G, C_OUT // G).sum(-1)  # (N, C, KT, G)
    Vsum3 = np.ascontiguousarray(Vsum3)
    WrT = np.ascontiguousarray(W_res.T)                  # (C, O)

    trivial_gn = bool(np.all(gng == 1.0) and np.all(gnb == 0.0))
    trivial_ln = bool(np.all(lng == 1.0) and np.all(lnb == 0.0))

    key = (trivial_gn, trivial_ln)
    if key not in _CACHE:
        _CACHE[key] = _build(*key)
    nc = _CACHE[key]

    # ---- shard nodes across cores ----
    splits = [6, 6, 6, 6, 6, 6, 6, 5]
    starts = np.cumsum([0] + splits[:-1])
    xt_full = np.zeros((N, C_IN, B, T + 2), np.float32)      # zero-padded t
    xt_full[:, :, :, 1 : T + 1] = x.transpose(2, 3, 0, 1)
    in_maps = []
    for c in range(NCORES):
        n0, nn = starts[c], splits[c]
        idx = list(range(n0, n0 + nn)) + [0] * (NN - nn)
        in_maps.append({
            "x_t": np.ascontiguousarray(xt_full[idx]),
            "v3a": np.ascontiguousarray(V3[idx, 0:128]),
            "v3b": np.ascontiguousarray(V3[idx, 128:192]),
            "v3sa": np.ascontiguousarray(Vsum3[idx, 0:128]),
            "v3sb": np.ascontiguousarray(Vsum3[idx, 128:192]),
            "eye3d": np.eye(GRP, dtype=np.float32),
            "wra": np.ascontiguousarray(WrT[0:128]),
            "wrb": np.ascontiguousarray(WrT[128:192]),
            "gng": gng.reshape(1, -1).copy(),
            "gnb": gnb.reshape(1, -1).copy(),
            "lng": lng.reshape(1, -1).copy(),
            "lnb": lnb.reshape(1, -1).copy(),
        })

    import time as _time
    _t0 = _time.perf_counter()
    res = run_bass_kernel_spmd(nc, in_maps, core_ids=list(range(NCORES)))
    global LAST_RUN_S
    LAST_RUN_S = _time.perf_counter() - _t0
    out = np.empty((B, T, N, C_OUT), np.float32)
    for c in range(NCORES):
        n0, nn = starts[c], splits[c]
        o = res.results[c]["out_t"]  # (NN, B, T, C_OUT)
        out[:, :, n0 : n0 + nn, :] = o[:nn].transpose(1, 2, 0, 3)
    return out

